# revision 19
# baseline (speedup 1.0000x reference)
"""Trainium2 Bass kernel for nn_LocalAttention (Luong local attention, N=64, L=H=1024).

Strategy
--------
Data-parallel over batch: 8 batches per NeuronCore x 8 cores.

Host-side layout prep (no model FLOPs on host):
  * For each batch n, p_t = max(src_len - time_step, -1). The Gaussian
    exp(-(l-p_t)^2/25) underflows to exactly 0.0f for |l-p_t| > 51, so the
    context reduction only needs a 128-wide window around p_t.
  * We ROLL each batch's source axis so that window lands at static slots
    [0, 128). Softmax (max/sum) is permutation-invariant, so scores/softmax
    computed in rolled coordinates are exact.
  * The full-L score stream is fp8(e4m3): it only feeds the softmax
    normalizer + max, which tolerate small score noise. The 128-wide window
    scores are recomputed exactly in fp16 from a small fp16 window tensor and
    OVERWRITE the fp8 ones in PSUM (start=True write mode), so the softmax
    numerator is fp16-accurate. Measured end-to-end rel err ~2e-4 (gate 2e-2).

Device per core (PSUM fp32 everywhere):
  qa = output @ W_a                       (PE fp16 stream, batched over 8 rows)
  qa^T via PE transposes; fp8 + fp16 copies
  per batch b (software-pipelined, ctx(b-1) emitted after scores(b)):
    scores = qa8_b . E8_b^T               (PE fp8 DoubleRow: K=256/instr)
    scores[0:128] = qa16_b . E16win_b^T   (PE fp16 overwrite, exact window)
    softmax on [1,1024] @ partition 0     (DVE max / ACT exp+sum / DVE)
    w = softmax * gauss / Z (window only) (DVE, one fused op)
    w^T via K=1 matmul with ones          (PE)
    ctx^T[h,b] = E16win_b-chunks @ w^T    (PE fp16, 8 tiny matmuls)
  OUT = tanh([ctx; output] @ W_c^T)       (PE fp16 batched over 8, ACT tanh)
"""

import os
import sys

import numpy as np
import ml_dtypes

for _p in ("/opt/trn_rl_repo", "/root/.axon_site/_ro/trn_rl_repo"):
    if os.path.isdir(_p) and _p not in sys.path:
        sys.path.insert(0, _p)

N, L, H = 64, 1024, 1024
NCORES = 8
NB = N // NCORES  # batches per core
WIN = 128         # static window width after roll
DEV_POW = 25.0
KC = H // 128     # 8 contraction chunks
PAIRS = KC // 2   # fp8 DoubleRow k-pairs

F8NP = ml_dtypes.float8_e4m3
F16NP = np.float16

_PROGRAM = None


def _build_program():
    import concourse.tile as tile
    from concourse import bacc, mybir
    from concourse.bass import MemorySpace, ts
    from concourse.masks import make_identity
    from contextlib import ExitStack

    F32 = mybir.dt.float32
    F16 = mybir.dt.float16
    F8 = mybir.dt.float8e4
    AF = mybir.ActivationFunctionType
    ALU = mybir.AluOpType
    DR = mybir.MatmulPerfMode.DoubleRow

    LT = L - WIN  # fp8 tail length (window cols excluded from the fp8 stream)
    nc = bacc.Bacc("TRN2", target_bir_lowering=False, debug=False, num_devices=NCORES)
    # eT8[b, p, pr, s, t] = E_rolled[b, WIN+t, 128*(2pr+s)+p] as fp8; 7KB/partition
    eT8 = nc.dram_tensor("eT8", [NB, 128, PAIRS, 2, LT], F8, kind="ExternalInput").ap()
    # eWinT[p, b, c, j] = E_rolled[b, j, 128c+p] fp16 (window cols on free)
    eWinT = nc.dram_tensor("eWinT", [128, NB, KC, WIN], F16, kind="ExternalInput").ap()
    # eWin[j, b, h] = E_rolled[b, j, h] fp16 (window rows on partitions)
    eWin = nc.dram_tensor("eWin", [WIN, NB, H], F16, kind="ExternalInput").ap()
    gaussW = nc.dram_tensor("gaussW", [1, NB, WIN], F32, kind="ExternalInput").ap()
    outT = nc.dram_tensor("outT", [128, KC, NB], F16, kind="ExternalInput").ap()
    wa = nc.dram_tensor("wa", [128, KC, H], F16, kind="ExternalInput").ap()
    wcT = nc.dram_tensor("wcT", [128, 2 * KC, H], F16, kind="ExternalInput").ap()
    res = nc.dram_tensor("res", [NB, H], F32, kind="ExternalOutput").ap()

    with tile.TileContext(nc) as tc, ExitStack() as ctx:
        consts = ctx.enter_context(tc.tile_pool(name="consts", bufs=1))
        etp = ctx.enter_context(tc.tile_pool(name="etp", bufs=3))
        work = ctx.enter_context(tc.tile_pool(name="work", bufs=2))
        ps_s = ctx.enter_context(
            tc.tile_pool(name="ps_s", bufs=2, space=MemorySpace.PSUM)
        )
        ps_m = ctx.enter_context(
            tc.tile_pool(name="ps_m", bufs=2, space=MemorySpace.PSUM)
        )

        # ---- constants / weights ----
        # Window tensors stream per-batch with lookahead; wcT streams in 1MB
        # chunks interleaved with the eT8 stream so the final projection never
        # waits on a serial 4MB tail DMA.
        outT_sb = consts.tile([128, KC, NB], F16)
        nc.sync.dma_start(outT_sb[:], outT[:])
        wa_sb = consts.tile([128, KC, H], F16)
        nc.sync.dma_start(wa_sb[:], wa[:])
        gauss_sb = consts.tile([1, NB, WIN], F32)
        nc.sync.dma_start(gauss_sb[:], gaussW[:])
        eWinT_sb = consts.tile([128, NB, KC, WIN], F16)
        eWin_sb = consts.tile([WIN, NB, H], F16)
        wcT_sb = consts.tile([128, 2 * KC, H], F16)

        def emit_win_dma(b):
            nc.sync.dma_start(eWinT_sb[:, b], eWinT[:, b])
            nc.sync.dma_start(eWin_sb[:, b], eWin[:, b])

        emit_win_dma(0)
        emit_win_dma(1)
        ident = consts.tile([128, 128], F32)
        make_identity(nc, ident[:])
        ones1 = consts.tile([1, 1], F32)
        nc.gpsimd.memset(ones1[:], 1.0)

        # ---- qa = output @ W_a (rows), then transpose to qa^T ----
        qa_sb = consts.tile([NB, H], F32)
        for hh in range(2):
            ps_qa = ps_m.tile([NB, 512], F32, tag="misc")
            for c in range(KC):
                nc.tensor.matmul(
                    ps_qa[:],
                    outT_sb[:, c, :],
                    wa_sb[:, c, ts(hh, 512)],
                    start=(c == 0),
                    stop=(c == KC - 1),
                )
            nc.vector.tensor_copy(qa_sb[:, ts(hh, 512)], ps_qa[:])
        qaT_sb = consts.tile([128, KC, NB], F32)
        for c in range(KC):
            ps_qaT = ps_m.tile([128, NB], F32, tag="misc")
            nc.tensor.transpose(ps_qaT[:], qa_sb[:, ts(c, 128)], ident[:NB, :NB])
            nc.vector.tensor_copy(qaT_sb[:, c, :], ps_qaT[:])
        # fp8 DoubleRow needs a full 128-wide stationary operand (small M fails
        # ISA validation): batch b's qa sits in column 0 of its slab, the rest
        # are zeros whose output rows are never read.
        qaT8_slab = consts.tile([128, NB, PAIRS, 2, 128], F8)
        nc.gpsimd.memset(qaT8_slab[:], 0.0)
        nc.vector.tensor_copy(
            qaT8_slab[:, :, :, :, 0],
            qaT_sb.rearrange("p (pr s) b -> p b pr s", s=2),
        )
        qaT16_sb = consts.tile([128, KC, NB], F16)
        nc.vector.tensor_copy(qaT16_sb[:], qaT_sb[:])

        ctxAll = consts.tile([128, KC, NB], F16)

        # ---- per-batch pipeline, software-pipelined on PE ----
        # scores layout in ps_sc row 0: [0:896) fp8 tail, [896:1024) fp16 window
        def emit_scores(b):
            et = etp.tile([128, PAIRS, 2, LT], F8, tag="et")
            nc.sync.dma_start(et[:], eT8[b])
            ps_sc = ps_s.tile([128, L], F32, tag="scores")
            for lo, nn in ((0, 512), (512, LT - 512)):
                for pr in range(PAIRS):
                    nc.tensor.matmul(
                        ps_sc[:, lo : lo + nn],
                        qaT8_slab[:, b, pr, :, :],
                        et[:, pr, :, lo : lo + nn],
                        start=(pr == 0),
                        stop=(pr == PAIRS - 1),
                        perf_mode=DR,
                    )
            # exact fp16 window scores in their own region
            for c in range(KC):
                nc.tensor.matmul(
                    ps_sc[0:1, LT:],
                    qaT16_sb[:, c, b : b + 1],
                    eWinT_sb[:, b, c, :],
                    start=(c == 0),
                    stop=(c == KC - 1),
                )
            # softmax chain on DVE/ACT (off the PE critical path)
            negmax = work.tile([1, 1], F32, tag="negmax")
            nc.vector.reduce_max(
                negmax[:], ps_sc[0:1, :], axis=mybir.AxisListType.X, negate=True
            )
            expv = work.tile([1, L], F32, tag="expv")
            zsum = work.tile([1, 1], F32, tag="zsum")
            nc.scalar.activation(
                expv[:], ps_sc[0:1, :], AF.Exp, bias=negmax[:], accum_out=zsum[:]
            )
            rz = work.tile([1, 1], F32, tag="rz")
            nc.vector.reciprocal(rz[:], zsum[:])
            wv = work.tile([1, WIN], F32, tag="wv")
            nc.vector.scalar_tensor_tensor(
                wv[:], expv[:, LT:], rz[:], gauss_sb[:, b, :],
                op0=ALU.mult, op1=ALU.mult,
            )
            return wv

        def emit_ctx(b, wv):
            ps_wT = ps_m.tile([WIN, 1], F32, tag="misc")
            nc.tensor.matmul(ps_wT[:], wv[:], ones1[:], start=True, stop=True)
            wT16 = work.tile([WIN, 1], F16, tag="wT16")
            nc.vector.tensor_copy(wT16[:], ps_wT[:])
            ps_ctx = ps_m.tile([128, KC], F32, tag="misc")
            for c in range(KC):
                nc.tensor.matmul(
                    ps_ctx[:, c : c + 1],
                    eWin_sb[:, b, ts(c, 128)],
                    wT16[:],
                    start=True,
                    stop=True,
                )
            nc.vector.tensor_copy(ctxAll[:, :, b], ps_ctx[:])

        wv_prev = None
        for b in range(NB):
            wv = emit_scores(b)
            if b + 2 < NB:
                emit_win_dma(b + 2)
            if b % 2 == 0:
                k = b // 2
                nc.sync.dma_start(wcT_sb[:, ts(k, 4), :], wcT[:, ts(k, 4), :])
            if wv_prev is not None:
                emit_ctx(b - 1, wv_prev)
            wv_prev = wv
        emit_ctx(NB - 1, wv_prev)

        # ---- OUT = tanh(cat @ W_c^T), batched over the core's 8 rows ----
        res_sb = work.tile([NB, H], F32, tag="res")
        for hh in range(2):
            ps_out = ps_m.tile([NB, 512], F32, tag="misc")
            for d in range(2 * KC):
                lhsT = ctxAll[:, d, :] if d < KC else outT_sb[:, d - KC, :]
                nc.tensor.matmul(
                    ps_out[:],
                    lhsT,
                    wcT_sb[:, d, ts(hh, 512)],
                    start=(d == 0),
                    stop=(d == 2 * KC - 1),
                )
            nc.scalar.activation(res_sb[:, ts(hh, 512)], ps_out[:], AF.Tanh)
        nc.sync.dma_start(res[:], res_sb[:])

    nc.compile()
    return nc


def _get_program():
    global _PROGRAM
    if _PROGRAM is None:
        _PROGRAM = _build_program()
    return _PROGRAM


def _prepare(inputs):
    E = np.asarray(inputs["encoder_outputs"], dtype=np.float32)
    out = np.asarray(inputs["output"], dtype=np.float32).reshape(N, H)
    W_a = np.ascontiguousarray(np.asarray(inputs["W_a"], dtype=np.float32))
    W_c = np.asarray(inputs["W_c"], dtype=np.float32)
    src_len = np.asarray(inputs["src_len"]).reshape(N).astype(np.int64)
    t = int(np.asarray(inputs["time_step"]))

    p_t = np.maximum(src_len - t, -1)
    roll = p_t - (WIN // 2 - 1)  # window slot j <-> original l = (j + roll) % L
    j = np.arange(L, dtype=np.int64)
    idx = (j[None, :] + roll[:, None]) % L  # (N, L)
    ptf = p_t.astype(np.float32)[:, None]
    gauss = np.exp(
        -((idx[:, :WIN].astype(np.float32) - ptf) ** 2) / np.float32(DEV_POW)
    ).astype(np.float32)  # (N, WIN): rolled gauss is 0 outside the window

    Er = E[np.arange(N)[:, None], idx, :]  # (N, L, H) rolled
    eT = Er.transpose(0, 2, 1)  # (N, H, L)
    # fp8 score stream, window columns excluded: [n, p, pr, s, t] = eT[n, h, WIN+t]
    eT8 = np.ascontiguousarray(
        eT[:, :, WIN:].reshape(N, PAIRS, 2, 128, L - WIN)
        .transpose(0, 3, 1, 2, 4)
        .astype(F8NP)
    )
    # fp16 window, h on partitions: [p, n, c, j] = eT[n, 128c+p, j<WIN]
    eWinT = eT[:, :, :WIN].reshape(N, KC, 128, WIN).transpose(2, 0, 1, 3).astype(F16NP)
    # fp16 window, l on partitions: [j, n, h] = Er[n, j<WIN, h]
    eWin = Er[:, :WIN, :].transpose(1, 0, 2).astype(F16NP)
    wa_dev = np.ascontiguousarray(
        W_a.reshape(KC, 128, H).transpose(1, 0, 2).astype(F16NP)
    )  # (128, KC, H)
    wcT_dev = np.ascontiguousarray(
        W_c.T.reshape(2 * KC, 128, H).transpose(1, 0, 2).astype(F16NP)
    )  # (128, 2KC, H)
    outT_all = out.T.reshape(KC, 128, N).transpose(1, 0, 2).astype(F16NP)

    in_maps = []
    for c in range(NCORES):
        sl = slice(c * NB, (c + 1) * NB)
        in_maps.append(
            {
                "eT8": eT8[sl],
                "eWinT": np.ascontiguousarray(eWinT[:, sl]),
                "eWin": np.ascontiguousarray(eWin[:, sl]),
                "gaussW": np.ascontiguousarray(gauss[sl])[None],
                "outT": np.ascontiguousarray(outT_all[:, :, sl]),
                "wa": wa_dev,
                "wcT": wcT_dev,
            }
        )
    return in_maps


def _run(inputs, trace=False, tmpdir=None):
    from concourse.bass_utils import run_bass_kernel_spmd

    nc = _get_program()
    in_maps = _prepare(inputs)
    r = run_bass_kernel_spmd(
        nc, in_maps, core_ids=list(range(NCORES)), trace=trace, tmpdir=tmpdir
    )
    outp = np.concatenate([r.results[c]["res"] for c in range(NCORES)], axis=0)
    return np.ascontiguousarray(outp.reshape(N, 1, H).astype(np.float32)), r


def kernel(**inputs):
    return _run(inputs, trace=False)[0]


# revision 23
# speedup vs baseline: 1.2343x; 1.2343x over previous
"""Trainium2 Bass kernel for nn_LocalAttention (Luong local attention, N=64, L=H=1024).

Strategy
--------
Data-parallel over batch: 8 batches per NeuronCore x 8 cores.

Host-side layout prep (no model FLOPs on host):
  * For each batch n, p_t = max(src_len - time_step, -1). The Gaussian
    exp(-(l-p_t)^2/25) underflows to exactly 0.0f for |l-p_t| > 51, so the
    context reduction only needs a 128-wide window around p_t.
  * We ROLL each batch's source axis so that window lands at static slots
    [0, 128). Softmax (max/sum) is permutation-invariant, so scores/softmax
    computed in rolled coordinates are exact.
  * The full-L score stream is fp8(e4m3): it only feeds the softmax
    normalizer + max, which tolerate small score noise. The 128-wide window
    scores are recomputed exactly in fp16 from a small fp16 window tensor and
    OVERWRITE the fp8 ones in PSUM (start=True write mode), so the softmax
    numerator is fp16-accurate. Measured end-to-end rel err ~2e-4 (gate 2e-2).

Device per core (PSUM fp32 everywhere):
  qa = output @ W_a                       (PE fp16 stream, batched over 8 rows)
  qa^T via PE transposes; fp8 + fp16 copies
  per batch b (software-pipelined, ctx(b-1) emitted after scores(b)):
    scores = qa8_b . E8_b^T               (PE fp8 DoubleRow: K=256/instr)
    scores[0:128] = qa16_b . E16win_b^T   (PE fp16 overwrite, exact window)
    softmax on [1,1024] @ partition 0     (DVE max / ACT exp+sum / DVE)
    w = softmax * gauss / Z (window only) (DVE, one fused op)
    w^T via K=1 matmul with ones          (PE)
    ctx^T[h,b] = E16win_b-chunks @ w^T    (PE fp16, 8 tiny matmuls)
  OUT = tanh([ctx; output] @ W_c^T)       (PE fp16 batched over 8, ACT tanh)
"""

import os
import sys

import numpy as np
import ml_dtypes

for _p in ("/opt/trn_rl_repo", "/root/.axon_site/_ro/trn_rl_repo"):
    if os.path.isdir(_p) and _p not in sys.path:
        sys.path.insert(0, _p)

N, L, H = 64, 1024, 1024
NCORES = 8
NB = N // NCORES  # batches per core
WIN = 128         # static window width after roll
DEV_POW = 25.0
KC = H // 128     # 8 contraction chunks
PAIRS = KC // 2   # fp8 DoubleRow k-pairs

F8NP = ml_dtypes.float8_e4m3
F16NP = np.float16

_PROGRAM = None


def _build_program():
    import concourse.tile as tile
    from concourse import bacc, mybir
    from concourse.bass import MemorySpace, ts
    from concourse.masks import make_identity
    from contextlib import ExitStack

    F32 = mybir.dt.float32
    F16 = mybir.dt.float16
    F8 = mybir.dt.float8e4
    AF = mybir.ActivationFunctionType
    ALU = mybir.AluOpType
    DR = mybir.MatmulPerfMode.DoubleRow

    LT = L - WIN  # fp8 tail length (window cols excluded from the fp8 stream)
    nc = bacc.Bacc("TRN2", target_bir_lowering=False, debug=False, num_devices=NCORES)
    # eT8[b, p, pr, s, t] = E_rolled[b, WIN+t, 128*(2pr+s)+p] as fp8; 7KB/partition
    eT8 = nc.dram_tensor("eT8", [NB, 128, PAIRS, 2, LT], F8, kind="ExternalInput").ap()
    # eWinT[p, b, c, j] = E_rolled[b, j, 128c+p] fp16 (window cols on free)
    eWinT = nc.dram_tensor("eWinT", [128, NB, KC, WIN], F16, kind="ExternalInput").ap()
    # eWin[j, b, h] = E_rolled[b, j, h] fp16 (window rows on partitions)
    eWin = nc.dram_tensor("eWin", [WIN, NB, H], F16, kind="ExternalInput").ap()
    gaussW = nc.dram_tensor("gaussW", [1, NB, WIN], F32, kind="ExternalInput").ap()
    outT = nc.dram_tensor("outT", [128, KC, NB], F16, kind="ExternalInput").ap()
    wa = nc.dram_tensor("wa", [128, KC, H], F16, kind="ExternalInput").ap()
    wcT = nc.dram_tensor("wcT", [128, 2 * KC, H], F16, kind="ExternalInput").ap()
    res = nc.dram_tensor("res", [NB, H], F32, kind="ExternalOutput").ap()

    with tile.TileContext(nc) as tc, ExitStack() as ctx:
        consts = ctx.enter_context(tc.tile_pool(name="consts", bufs=1))
        etp = ctx.enter_context(tc.tile_pool(name="etp", bufs=3))
        work = ctx.enter_context(tc.tile_pool(name="work", bufs=2))
        ps_s = ctx.enter_context(
            tc.tile_pool(name="ps_s", bufs=2, space=MemorySpace.PSUM)
        )
        ps_m = ctx.enter_context(
            tc.tile_pool(name="ps_m", bufs=2, space=MemorySpace.PSUM)
        )

        # ---- constants / weights ----
        # Two independent HW DMA dispatch queues: the SP queue (nc.sync)
        # carries the latency-critical fp8 eT8 stream; the Activation queue
        # (nc.scalar) carries everything else so the stream is never
        # head-of-line blocked behind weights/window transfers.
        outT_sb = consts.tile([128, KC, NB], F16)
        nc.sync.dma_start(outT_sb[:], outT[:])
        gauss_sb = consts.tile([1, NB, WIN], F32)
        nc.sync.dma_start(gauss_sb[:], gaussW[:])
        wa_sb = consts.tile([128, KC, H], F16)
        nc.scalar.dma_start(wa_sb[:, :4], wa[:, :4])
        nc.scalar.dma_start(wa_sb[:, 4:], wa[:, 4:])
        eWinT_sb = consts.tile([128, NB, KC, WIN], F16)
        eWin_sb = consts.tile([WIN, NB, H], F16)
        for b in range(NB):
            nc.scalar.dma_start(eWinT_sb[:, b], eWinT[:, b])
            nc.scalar.dma_start(eWin_sb[:, b], eWin[:, b])
        wcT_sb = consts.tile([128, 2 * KC, H], F16)
        for k in range(4):
            nc.scalar.dma_start(wcT_sb[:, ts(k, 4), :], wcT[:, ts(k, 4), :])
        ident = consts.tile([128, 128], F32)
        make_identity(nc, ident[:])
        ones1 = consts.tile([1, 1], F32)
        nc.gpsimd.memset(ones1[:], 1.0)

        # ---- qa = output @ W_a (rows), then transpose to qa^T ----
        qa_sb = consts.tile([NB, H], F32)
        for hh in range(2):
            ps_qa = ps_m.tile([NB, 512], F32, tag="misc")
            for c in range(KC):
                nc.tensor.matmul(
                    ps_qa[:],
                    outT_sb[:, c, :],
                    wa_sb[:, c, ts(hh, 512)],
                    start=(c == 0),
                    stop=(c == KC - 1),
                )
            nc.vector.tensor_copy(qa_sb[:, ts(hh, 512)], ps_qa[:])
        qaT_sb = consts.tile([128, KC, NB], F32)
        for c in range(KC):
            ps_qaT = ps_m.tile([128, NB], F32, tag="misc")
            nc.tensor.transpose(ps_qaT[:], qa_sb[:, ts(c, 128)], ident[:NB, :NB])
            nc.vector.tensor_copy(qaT_sb[:, c, :], ps_qaT[:])
        # fp8 DoubleRow rejects tiny stationary free sizes (M=1/8 fail ISA
        # validation, M>=16 passes): batch b's qa sits in column 0 of its
        # 16-wide slab, the rest are zeros whose output rows are never read.
        MSLAB = 16
        qaT8_slab = consts.tile([128, NB, PAIRS, 2, MSLAB], F8)
        nc.gpsimd.memset(qaT8_slab[:], 0.0)
        nc.vector.tensor_copy(
            qaT8_slab[:, :, :, :, 0],
            qaT_sb.rearrange("p (pr s) b -> p b pr s", s=2),
        )
        qaT16_sb = consts.tile([128, KC, NB], F16)
        nc.vector.tensor_copy(qaT16_sb[:], qaT_sb[:])

        ctxAll = consts.tile([128, KC, NB], F16)

        # ---- per-batch pipeline, software-pipelined on PE ----
        # scores layout in ps_sc row 0: [0:896) fp8 tail, [896:1024) fp16 window
        def emit_scores(b):
            et = etp.tile([128, PAIRS, 2, LT], F8, tag="et")
            nc.sync.dma_start(et[:], eT8[b])
            ps_sc = ps_s.tile([MSLAB, L], F32, tag="scores")
            for lo, nn in ((0, 512), (512, LT - 512)):
                for pr in range(PAIRS):
                    nc.tensor.matmul(
                        ps_sc[:, lo : lo + nn],
                        qaT8_slab[:, b, pr, :, :],
                        et[:, pr, :, lo : lo + nn],
                        start=(pr == 0),
                        stop=(pr == PAIRS - 1),
                        perf_mode=DR,
                    )
            # exact fp16 window scores in their own region
            for c in range(KC):
                nc.tensor.matmul(
                    ps_sc[0:1, LT:],
                    qaT16_sb[:, c, b : b + 1],
                    eWinT_sb[:, b, c, :],
                    start=(c == 0),
                    stop=(c == KC - 1),
                )
            # softmax chain on DVE/ACT (off the PE critical path)
            negmax = work.tile([1, 1], F32, tag="negmax")
            nc.vector.reduce_max(
                negmax[:], ps_sc[0:1, :], axis=mybir.AxisListType.X, negate=True
            )
            expv = work.tile([1, L], F32, tag="expv")
            zsum = work.tile([1, 1], F32, tag="zsum")
            nc.scalar.activation(
                expv[:], ps_sc[0:1, :], AF.Exp, bias=negmax[:], accum_out=zsum[:]
            )
            rz = work.tile([1, 1], F32, tag="rz")
            nc.vector.reciprocal(rz[:], zsum[:])
            wv = work.tile([1, WIN], F32, tag="wv")
            nc.vector.scalar_tensor_tensor(
                wv[:], expv[:, LT:], rz[:], gauss_sb[:, b, :],
                op0=ALU.mult, op1=ALU.mult,
            )
            return wv

        def emit_ctx(b, wv):
            ps_wT = ps_m.tile([WIN, 1], F32, tag="misc")
            nc.tensor.matmul(ps_wT[:], wv[:], ones1[:], start=True, stop=True)
            wT16 = work.tile([WIN, 1], F16, tag="wT16")
            nc.vector.tensor_copy(wT16[:], ps_wT[:])
            ps_ctx = ps_m.tile([128, KC], F32, tag="misc")
            for c in range(KC):
                nc.tensor.matmul(
                    ps_ctx[:, c : c + 1],
                    eWin_sb[:, b, ts(c, 128)],
                    wT16[:],
                    start=True,
                    stop=True,
                )
            nc.vector.tensor_copy(ctxAll[:, :, b], ps_ctx[:])

        wv_prev = None
        for b in range(NB):
            wv = emit_scores(b)
            if wv_prev is not None:
                emit_ctx(b - 1, wv_prev)
            wv_prev = wv
        emit_ctx(NB - 1, wv_prev)

        # ---- OUT = tanh(cat @ W_c^T), batched over the core's 8 rows ----
        res_sb = work.tile([NB, H], F32, tag="res")
        for hh in range(2):
            ps_out = ps_m.tile([NB, 512], F32, tag="misc")
            for d in range(2 * KC):
                lhsT = ctxAll[:, d, :] if d < KC else outT_sb[:, d - KC, :]
                nc.tensor.matmul(
                    ps_out[:],
                    lhsT,
                    wcT_sb[:, d, ts(hh, 512)],
                    start=(d == 0),
                    stop=(d == 2 * KC - 1),
                )
            nc.scalar.activation(res_sb[:, ts(hh, 512)], ps_out[:], AF.Tanh)
        nc.sync.dma_start(res[:], res_sb[:])

    nc.compile()
    return nc


def _get_program():
    global _PROGRAM
    if _PROGRAM is None:
        _PROGRAM = _build_program()
    return _PROGRAM


def _prepare(inputs):
    E = np.asarray(inputs["encoder_outputs"], dtype=np.float32)
    out = np.asarray(inputs["output"], dtype=np.float32).reshape(N, H)
    W_a = np.ascontiguousarray(np.asarray(inputs["W_a"], dtype=np.float32))
    W_c = np.asarray(inputs["W_c"], dtype=np.float32)
    src_len = np.asarray(inputs["src_len"]).reshape(N).astype(np.int64)
    t = int(np.asarray(inputs["time_step"]))

    p_t = np.maximum(src_len - t, -1)
    roll = p_t - (WIN // 2 - 1)  # window slot j <-> original l = (j + roll) % L
    j = np.arange(L, dtype=np.int64)
    idx = (j[None, :] + roll[:, None]) % L  # (N, L)
    ptf = p_t.astype(np.float32)[:, None]
    gauss = np.exp(
        -((idx[:, :WIN].astype(np.float32) - ptf) ** 2) / np.float32(DEV_POW)
    ).astype(np.float32)  # (N, WIN): rolled gauss is 0 outside the window

    Er = E[np.arange(N)[:, None], idx, :]  # (N, L, H) rolled
    eT = Er.transpose(0, 2, 1)  # (N, H, L)
    # fp8 score stream, window columns excluded: [n, p, pr, s, t] = eT[n, h, WIN+t]
    eT8 = np.ascontiguousarray(
        eT[:, :, WIN:].reshape(N, PAIRS, 2, 128, L - WIN)
        .transpose(0, 3, 1, 2, 4)
        .astype(F8NP)
    )
    # fp16 window, h on partitions: [p, n, c, j] = eT[n, 128c+p, j<WIN]
    eWinT = eT[:, :, :WIN].reshape(N, KC, 128, WIN).transpose(2, 0, 1, 3).astype(F16NP)
    # fp16 window, l on partitions: [j, n, h] = Er[n, j<WIN, h]
    eWin = Er[:, :WIN, :].transpose(1, 0, 2).astype(F16NP)
    wa_dev = np.ascontiguousarray(
        W_a.reshape(KC, 128, H).transpose(1, 0, 2).astype(F16NP)
    )  # (128, KC, H)
    wcT_dev = np.ascontiguousarray(
        W_c.T.reshape(2 * KC, 128, H).transpose(1, 0, 2).astype(F16NP)
    )  # (128, 2KC, H)
    outT_all = out.T.reshape(KC, 128, N).transpose(1, 0, 2).astype(F16NP)

    in_maps = []
    for c in range(NCORES):
        sl = slice(c * NB, (c + 1) * NB)
        in_maps.append(
            {
                "eT8": eT8[sl],
                "eWinT": np.ascontiguousarray(eWinT[:, sl]),
                "eWin": np.ascontiguousarray(eWin[:, sl]),
                "gaussW": np.ascontiguousarray(gauss[sl])[None],
                "outT": np.ascontiguousarray(outT_all[:, :, sl]),
                "wa": wa_dev,
                "wcT": wcT_dev,
            }
        )
    return in_maps


def _run(inputs, trace=False, tmpdir=None):
    from concourse.bass_utils import run_bass_kernel_spmd

    nc = _get_program()
    in_maps = _prepare(inputs)
    r = run_bass_kernel_spmd(
        nc, in_maps, core_ids=list(range(NCORES)), trace=trace, tmpdir=tmpdir
    )
    outp = np.concatenate([r.results[c]["res"] for c in range(NCORES)], axis=0)
    return np.ascontiguousarray(outp.reshape(N, 1, H).astype(np.float32)), r


def kernel(**inputs):
    return _run(inputs, trace=False)[0]
